# revision 2
# baseline (speedup 1.0000x reference)
"""Task-conditional linear (MoE routing) Bass kernel for TRN2.

Reference computation:
    w = W[task_id].reshape(B, 2*fout, fin)          # [256, 1024, 1024] gather
    logits = einsum('boi,bi->bo', w, x)             # per-sample matvec
    h = 1 - argmax(logits.reshape(B, fout, 2), -1)  # pairwise compare

Strategy: only 20 distinct weight matrices exist, so group samples by task
and read each task's [1024, 1024] matrix from HBM exactly once (84 MB total
vs 1 GB for the naive per-sample gather).  Work is split into units of
(task sample-block <=32, fout-tile of 512): 40 units for the typical
distribution, balanced 5 per core across 8 NeuronCores -> ~10.5 MB of
weights per core, which pins the kernel at the DMA roofline (~30 us).

Per unit the PE computes  out[s, o] = sum_i xT[i, s] * WT[i, o]  as 8
accumulating matmuls (K=128 chunks of fin), lhsT = xT block (stationary),
rhs = WT tile (moving, N=512 = one PSUM bank).  h is computed on-device by
a stride-2 VectorE compare over the PSUM logits.  W is pre-transposed on
the host (layout prep, done once per call) so every DMA is 2 KB-contiguous.
"""

import os

import numpy as np

B, FIN, FOUT2, N_TASKS = 256, 1024, 1024, 20
P = 32            # padded samples per block (max task count is ~19 for B=256/20 tasks)
O_TILE = 512      # fout elements per unit = max fp32 matmul free dim = 1 PSUM bank
N_CORES = 8
KP = 128          # contraction chunk = partition count
KC = FIN // KP    # 8 k-chunks

_NC_CACHE: dict = {}

LAST_EXEC_TIME_NS = None
LAST_TRACE = None


def _build_nc(n_units: int):
    """Build + compile the SPMD Tile program for n_units units per core."""
    import concourse.mybir as mybir
    import concourse.tile as tile
    from concourse import bacc

    f32 = mybir.dt.float32
    nc = bacc.Bacc(
        "TRN2", target_bir_lowering=False, debug=False, num_devices=N_CORES
    )
    wt_d = nc.dram_tensor("wt", [n_units, FIN, O_TILE], f32, kind="ExternalInput")
    xt_d = nc.dram_tensor("xt", [n_units, FIN, P], f32, kind="ExternalInput")
    lo_d = nc.dram_tensor("lo", [n_units, P, O_TILE], f32, kind="ExternalOutput")
    ho_d = nc.dram_tensor("ho", [n_units, P, O_TILE // 2], f32, kind="ExternalOutput")

    with tile.TileContext(nc) as tc:
        with (
            tc.tile_pool(name="wts", bufs=3) as wpool,
            tc.tile_pool(name="xts", bufs=3) as xpool,
            tc.tile_pool(name="ps", bufs=4, space="PSUM") as pspool,
            tc.tile_pool(name="outs", bufs=4) as opool,
        ):
            for u in range(n_units):
                wt_sb = wpool.tile([KP, KC, O_TILE], f32, tag="w")
                nc.sync.dma_start(
                    wt_sb[:], wt_d[u].rearrange("(kc p) o -> p kc o", p=KP)
                )
                xt_sb = xpool.tile([KP, KC, P], f32, tag="x")
                nc.sync.dma_start(
                    xt_sb[:], xt_d[u].rearrange("(kc p) s -> p kc s", p=KP)
                )
                ps = pspool.tile([P, O_TILE], f32, tag="ps")
                for kc in range(KC):
                    nc.tensor.matmul(
                        ps[:],
                        xt_sb[:, kc, :],
                        wt_sb[:, kc, :],
                        start=(kc == 0),
                        stop=(kc == KC - 1),
                    )
                lo_sb = opool.tile([P, O_TILE], f32, tag="lo")
                nc.scalar.copy(lo_sb[:], ps[:])
                ho_sb = opool.tile([P, O_TILE // 2], f32, tag="ho")
                lo3 = lo_sb[:].rearrange("m (o two) -> m o two", two=2)
                nc.vector.tensor_tensor(
                    ho_sb[:], lo3[:, :, 0], lo3[:, :, 1], mybir.AluOpType.is_ge
                )
                nc.sync.dma_start(lo_d[u], lo_sb[:])
                nc.sync.dma_start(ho_d[u], ho_sb[:])

    nc.compile()
    return nc


def _get_nc(n_units: int):
    if n_units not in _NC_CACHE:
        _NC_CACHE[n_units] = _build_nc(n_units)
    return _NC_CACHE[n_units]


def kernel(x, task_id, W):
    global LAST_EXEC_TIME_NS, LAST_TRACE
    from concourse.bass_utils import run_bass_kernel_spmd

    x = np.ascontiguousarray(np.asarray(x), dtype=np.float32)
    task_id = np.asarray(task_id).astype(np.int64)
    W = np.asarray(W)
    b, fin = x.shape
    n_tasks = W.shape[0]
    fout2 = W.shape[1] // fin
    assert (b, fin, fout2) == (B, FIN, FOUT2), "kernel compiled for fixed shapes"
    Wr = W.reshape(n_tasks, fout2, fin)

    # --- shard: units of (task sample-block, fout tile) -----------------
    units = []  # (task, oh, sample_indices)
    for t in range(n_tasks):
        idx = np.nonzero(task_id == t)[0]
        for s in range(0, len(idx), P):
            blk = idx[s : s + P]
            for oh in range(fout2 // O_TILE):
                units.append((t, oh, blk))
    U = -(-len(units) // N_CORES)

    assign = [units[c::N_CORES] for c in range(N_CORES)]
    empty = np.empty(0, np.int64)
    for c in range(N_CORES):
        while len(assign[c]) < U:
            assign[c].append((0, 0, empty))  # dummy unit, output ignored

    in_maps = []
    for c in range(N_CORES):
        wt = np.empty((U, FIN, O_TILE), np.float32)
        xt = np.zeros((U, FIN, P), np.float32)
        for u, (t, oh, blk) in enumerate(assign[c]):
            wt[u] = Wr[t, oh * O_TILE : (oh + 1) * O_TILE, :].T
            if len(blk):
                xt[u, :, : len(blk)] = x[blk].T
        in_maps.append({"wt": wt, "xt": xt})

    # --- run on 8 cores -------------------------------------------------
    nc = _get_nc(U)
    trace = os.environ.get("BASS_KERNEL_TRACE") == "1"
    kw = {}
    if trace:
        kw = dict(trace=True, trace_cores=list(range(N_CORES)))
    res = run_bass_kernel_spmd(nc, in_maps, list(range(N_CORES)), **kw)
    LAST_EXEC_TIME_NS = res.exec_time_ns
    LAST_TRACE = res.instructions_and_trace

    # --- unshard ---------------------------------------------------------
    logits = np.empty((B, FOUT2), np.float32)
    h = np.empty((B, FOUT2 // 2), np.float32)
    OH = O_TILE // 2
    for c in range(N_CORES):
        lo = res.results[c]["lo"]
        ho = res.results[c]["ho"]
        for u, (t, oh, blk) in enumerate(assign[c]):
            n = len(blk)
            if n:
                logits[blk, oh * O_TILE : (oh + 1) * O_TILE] = lo[u, :n]
                h[blk, oh * OH : (oh + 1) * OH] = ho[u, :n]
    return h, logits.reshape(B, FOUT2 // 2, 2)


# revision 8
# speedup vs baseline: 1.0987x; 1.0987x over previous
"""Task-conditional linear (MoE routing) Bass kernel for TRN2.

Reference computation:
    w = W[task_id].reshape(B, 2*fout, fin)          # [256, 1024, 1024] gather
    logits = einsum('boi,bi->bo', w, x)             # per-sample matvec
    h = 1 - argmax(logits.reshape(B, fout, 2), -1)  # pairwise compare

Strategy: only 20 distinct weight matrices exist, so group samples by task
and read each task's [1024, 1024] matrix from HBM exactly once (84 MB total
vs 1 GB for the naive per-sample gather).  Work is split into units of
(task sample-block <=32, fout-tile of 512): 40 units for the typical
distribution, balanced 5 per core across 8 NeuronCores -> ~10.5 MB of
weights per core, which pins the kernel at the DMA/PE roofline.

Per unit the PE computes  out[s, o] = sum_i xT[i, s] * WT[i, o]  as 8
accumulating matmuls (K=128 chunks of fin), lhsT = xT block (stationary),
rhs = WT tile (moving, N=512 = one PSUM bank).  Matmuls run in full fp32
(2-pass); float32r would stream 4x faster but rounds mantissas to 11 bits
(~2e-4 rel err), which is not acceptable against an fp32 reference with
logit pair gaps down to 3e-5.  All x blocks load upfront in one DMA;
outputs go out on the SWDGE queue so the Sync HWDGE FIFO streams only
weights, split into pieces so the PE starts as soon as the first piece
lands.  h is computed on-device by a stride-2 VectorE compare.
W is pre-transposed on the host (layout prep) so every DMA is 2 KB-contiguous.
"""

import os

import numpy as np

B, FIN, FOUT2, N_TASKS = 256, 1024, 1024, 20
P = 32            # padded samples per block (max task count is ~19 for B=256/20 tasks)
O_TILE = 512      # fout elements per unit = max fp32 matmul free dim = 1 PSUM bank
N_CORES = 8
KP = 128          # contraction chunk = partition count
KC = FIN // KP    # 8 k-chunks

_NC_CACHE: dict = {}

LAST_EXEC_TIME_NS = None
LAST_MEAN_NS = None
LAST_TRACE = None


def _build_nc(n_units: int, fp32r: bool = True):
    """Build + compile the SPMD Tile program for n_units units per core."""
    import concourse.mybir as mybir
    import concourse.tile as tile
    from concourse import bacc

    f32 = mybir.dt.float32
    mm_dt = mybir.dt.float32r if fp32r else f32
    nc = bacc.Bacc(
        "TRN2", target_bir_lowering=False, debug=False, num_devices=N_CORES
    )
    wt_d = nc.dram_tensor("wt", [n_units, FIN, O_TILE], f32, kind="ExternalInput")
    # xt layout: [FIN, n_units * P] so the single upfront DMA has long
    # contiguous runs (n_units*P*4 bytes per fin row)
    xt_d = nc.dram_tensor("xt", [FIN, n_units * P], f32, kind="ExternalInput")
    lo_d = nc.dram_tensor("lo", [n_units, P, O_TILE], f32, kind="ExternalOutput")
    ho_d = nc.dram_tensor("ho", [n_units, P, O_TILE // 2], f32, kind="ExternalOutput")

    with tile.TileContext(nc) as tc:
        with (
            tc.tile_pool(name="xts", bufs=1) as xpool,
            tc.tile_pool(name="wts", bufs=6) as wpool,
            tc.tile_pool(name="ps", bufs=4, space="PSUM") as pspool,
            tc.tile_pool(name="outs", bufs=4) as opool,
        ):
            # all x blocks upfront, one DMA on the gpsimd (SWDGE) queue so it
            # never waits behind the big weight streams on the Sync FIFO
            xt_sb = xpool.tile([KP, KC, n_units * P], f32)
            nc.gpsimd.dma_start(
                xt_sb[:], xt_d.rearrange("(kc p) s -> p kc s", p=KP)
            )
            for u in range(n_units):
                # finer pieces for unit 0 so the PE starts as early as
                # possible; 1 MB pieces afterwards (DMA efficiency)
                kc_split = 2 if u == 0 else 4
                pieces = []
                for k0 in range(0, KC, kc_split):
                    wt_sb = wpool.tile([KP, kc_split, O_TILE], f32, tag="w")
                    nc.sync.dma_start(
                        wt_sb[:],
                        wt_d[u, k0 * KP : (k0 + kc_split) * KP, :].rearrange(
                            "(kc p) o -> p kc o", p=KP
                        ),
                    )
                    pieces.append((k0, wt_sb))
                ps = pspool.tile([P, O_TILE], f32, tag="ps")
                for k0, wt_sb in pieces:
                    for j in range(kc_split):
                        kc = k0 + j
                        nc.tensor.matmul(
                            ps[:],
                            xt_sb[:, kc, u * P : (u + 1) * P].bitcast(mm_dt),
                            wt_sb[:, j, :].bitcast(mm_dt),
                            start=(kc == 0),
                            stop=(kc == KC - 1),
                        )
                lo_sb = opool.tile([P, O_TILE], f32, tag="lo")
                nc.vector.tensor_copy(lo_sb[:], ps[:])
                ho_sb = opool.tile([P, O_TILE // 2], f32, tag="ho")
                lo3 = lo_sb[:].rearrange("m (o two) -> m o two", two=2)
                nc.vector.tensor_tensor(
                    ho_sb[:], lo3[:, :, 0], lo3[:, :, 1], mybir.AluOpType.is_ge
                )
                nc.gpsimd.dma_start(lo_d[u], lo_sb[:])
                nc.gpsimd.dma_start(ho_d[u], ho_sb[:])

    nc.compile()
    return nc


def _get_nc(n_units: int, fp32r: bool):
    key = (n_units, fp32r)
    if key not in _NC_CACHE:
        _NC_CACHE[key] = _build_nc(n_units, fp32r)
    return _NC_CACHE[key]


def kernel(x, task_id, W):
    global LAST_EXEC_TIME_NS, LAST_MEAN_NS, LAST_TRACE
    from concourse.bass_utils import run_bass_kernel_spmd

    x = np.ascontiguousarray(np.asarray(x), dtype=np.float32)
    task_id = np.asarray(task_id).astype(np.int64)
    W = np.asarray(W)
    b, fin = x.shape
    n_tasks = W.shape[0]
    fout2 = W.shape[1] // fin
    assert (b, fin, fout2) == (B, FIN, FOUT2), "kernel compiled for fixed shapes"
    Wr = W.reshape(n_tasks, fout2, fin)

    # --- shard: units of (task sample-block, fout tile) -----------------
    units = []  # (task, oh, sample_indices)
    for t in range(n_tasks):
        idx = np.nonzero(task_id == t)[0]
        for s in range(0, len(idx), P):
            blk = idx[s : s + P]
            for oh in range(fout2 // O_TILE):
                units.append((t, oh, blk))
    U = -(-len(units) // N_CORES)

    assign = [units[c::N_CORES] for c in range(N_CORES)]
    empty = np.empty(0, np.int64)
    for c in range(N_CORES):
        while len(assign[c]) < U:
            assign[c].append((0, 0, empty))  # dummy unit, output ignored

    in_maps = []
    for c in range(N_CORES):
        wt = np.empty((U, FIN, O_TILE), np.float32)
        xt = np.zeros((FIN, U * P), np.float32)
        for u, (t, oh, blk) in enumerate(assign[c]):
            wt[u] = Wr[t, oh * O_TILE : (oh + 1) * O_TILE, :].T
            if len(blk):
                xt[:, u * P : u * P + len(blk)] = x[blk].T
        in_maps.append({"wt": wt, "xt": xt})

    # --- run on 8 cores -------------------------------------------------
    fp32r = os.environ.get("BASS_FP32R", "0") == "1"
    nc = _get_nc(U, fp32r)
    trace = os.environ.get("BASS_KERNEL_TRACE") == "1"
    kw = {}
    if trace:
        kw = dict(trace=True, trace_cores=list(range(N_CORES)))
    res = run_bass_kernel_spmd(nc, in_maps, list(range(N_CORES)), **kw)
    LAST_EXEC_TIME_NS = res.exec_time_ns
    LAST_MEAN_NS = res.mean_exec_time_ns
    LAST_TRACE = res.instructions_and_trace

    # --- unshard ---------------------------------------------------------
    logits = np.empty((B, FOUT2), np.float32)
    h = np.empty((B, FOUT2 // 2), np.float32)
    OH = O_TILE // 2
    for c in range(N_CORES):
        lo = res.results[c]["lo"]
        ho = res.results[c]["ho"]
        for u, (t, oh, blk) in enumerate(assign[c]):
            n = len(blk)
            if n:
                logits[blk, oh * O_TILE : (oh + 1) * O_TILE] = lo[u, :n]
                h[blk, oh * OH : (oh + 1) * OH] = ho[u, :n]
    return h, logits.reshape(B, FOUT2 // 2, 2)
